# revision 2
# baseline (speedup 1.0000x reference)
"""Trainium2 Bass kernel for BarycentricCoordinates (retrieval_knn) — v2.

Per (v, r) problem: nearest-neighbor ordering of 8 projected points vs a
template vertex, barycentric weights for every (second, third) pair,
Delaunay empty-circumcircle filter, min-score pair selection.

v2 redesign vs baseline:
 - Delaunay dets depend on r ONLY through the closest index c(r) in 0..7:
   computed once per v-tile at (ij=64, c=8, k=8) instead of per-r
   (5x less work), reduced over k, and packed into an 8-bit-per-pair
   validity table okall(ij) = sum_c validc * 2^c.  Per r the bit is
   extracted with okall * 2^-c(r) -> int -> &1.
 - valid pairs have w0+w1+w2 = 1 with all weights > 0 => score
   max(w^2) = (max w)^2, so argmin(max w) == argmin(max w^2): no squares.
 - single r-chunk (all 40 at once), reciprocal_approx_accurate (2 ULP),
   scalar_tensor_tensor / tensor_scalar fusions (tensor_scalar runs 2x on
   DVE even in fp32), one-hot argmin with value-equality one-hot.
 - w1(i,j) == w2(j,i) bitwise -> transposed views, single gather pair.
 - 8 cores data-parallel over V.
"""

import sys

sys.path.insert(0, "/opt/trn_rl_repo")

import numpy as np

import concourse.bass as bass
import concourse.bacc as bacc
import concourse.mybir as mybir
from concourse.tile import TileContext

F32 = mybir.dt.float32
I32 = mybir.dt.int32
I16 = mybir.dt.int16
OP = mybir.AluOpType
AF = mybir.ActivationFunctionType
AX = mybir.AxisListType

BIG = 2.0e38
SB = 1.0e6          # small-big offset for index packing
N_CORES = 8
V_TOTAL = 5000
R, A, K0 = 5, 8, 8
RA = R * A          # 40
VS = V_TOTAL // N_CORES
P = 128
VSP = 640
K2 = 64
RK = RA * K0        # 320
PP = RA * K2        # 2560
DET = K2 * K0 * K0  # 4096


def build_nc(vsp=VSP):
    nc = bacc.Bacc("TRN2", target_bir_lowering=False)
    n_vt = vsp // P

    px_d = nc.dram_tensor("px", (vsp, K0), F32, kind="ExternalInput")
    py_d = nc.dram_tensor("py", (vsp, K0), F32, kind="ExternalInput")
    tmpl_d = nc.dram_tensor("tmpl", (2, RA), F32, kind="ExternalInput")
    neqp_d = nc.dram_tensor("neqp", (1, 512), F32, kind="ExternalInput")
    iota8_d = nc.dram_tensor("iota8", (1, K0), F32, kind="ExternalInput")
    p2neg_d = nc.dram_tensor("p2neg", (1, K0), F32, kind="ExternalInput")
    iota64b_d = nc.dram_tensor("iota64b", (1, K2), F32, kind="ExternalInput")
    outw_d = nc.dram_tensor("outw", (vsp, RA, 3), F32, kind="ExternalOutput")
    outi_d = nc.dram_tensor("outi", (vsp, RA, 3), F32, kind="ExternalOutput")

    with TileContext(nc) as tc:
        VE = nc.vector
        GP = nc.gpsimd
        SC = nc.scalar

        def bcv(ap, shape):
            return ap.to_broadcast(shape)

        with (
            tc.tile_pool(name="const", bufs=1) as cpool,
            tc.tile_pool(name="vt", bufs=2) as vpool,
            tc.tile_pool(name="det", bufs=1) as dpool,
            tc.tile_pool(name="ij", bufs=1) as ipool,
            tc.tile_pool(name="rk", bufs=1) as rkpool,
            tc.tile_pool(name="small", bufs=2) as opool,
        ):
            TX = cpool.tile([P, RA], F32, tag="TX")
            TY = cpool.tile([P, RA], F32, tag="TY")
            NEQP = cpool.tile([P, 512], F32, tag="NEQP")
            IOTA8 = cpool.tile([P, K0], F32, tag="IOTA8")
            P2NEG = cpool.tile([P, K0], F32, tag="P2NEG")
            IOTA64B = cpool.tile([P, K2], F32, tag="IOTA64B")
            nc.sync.dma_start(TX, tmpl_d[0:1, :].to_broadcast((P, RA)))
            nc.sync.dma_start(TY, tmpl_d[1:2, :].to_broadcast((P, RA)))
            nc.sync.dma_start(NEQP, neqp_d[0:1, :].to_broadcast((P, 512)))
            nc.sync.dma_start(IOTA8, iota8_d[0:1, :].to_broadcast((P, K0)))
            nc.sync.dma_start(P2NEG, p2neg_d[0:1, :].to_broadcast((P, K0)))
            nc.sync.dma_start(IOTA64B, iota64b_d[0:1, :].to_broadcast((P, K2)))

            for vt in range(n_vt):
                v0_, v1_ = vt * P, (vt + 1) * P
                px = vpool.tile([P, K0], F32, tag="px")
                py = vpool.tile([P, K0], F32, tag="py")
                nc.sync.dma_start(px, px_d[v0_:v1_, :])
                nc.sync.dma_start(py, py_d[v0_:v1_, :])

                # ---- s = |p|^2 ----
                t8a = vpool.tile([P, K0], F32, tag="t8a")
                t8b = vpool.tile([P, K0], F32, tag="t8b")
                s_ = vpool.tile([P, K0], F32, tag="s")
                SC.activation(out=t8a, in_=px, func=AF.Square)
                SC.activation(out=t8b, in_=py, func=AF.Square)
                VE.tensor_tensor(out=s_, in0=t8a, in1=t8b, op=OP.add)

                # ---- b-tensors b(c,k) = p_c - p_k (also used as (i,k)/(j,k)) ----
                bx = vpool.tile([P, K2], F32, tag="bx")
                by = vpool.tile([P, K2], F32, tag="by")
                bs = vpool.tile([P, K2], F32, tag="bs")
                bxv = bx.rearrange("p (i k) -> p i k", k=K0)
                byv = by.rearrange("p (i k) -> p i k", k=K0)
                bsv = bs.rearrange("p (i k) -> p i k", k=K0)
                VE.tensor_tensor(out=bxv, in0=bcv(px.unsqueeze(2), (P, K0, K0)),
                                 in1=bcv(px.unsqueeze(1), (P, K0, K0)), op=OP.subtract)
                GP.tensor_tensor(out=byv, in0=bcv(py.unsqueeze(2), (P, K0, K0)),
                                 in1=bcv(py.unsqueeze(1), (P, K0, K0)), op=OP.subtract)
                VE.tensor_tensor(out=bsv, in0=bcv(s_.unsqueeze(2), (P, K0, K0)),
                                 in1=bcv(s_.unsqueeze(1), (P, K0, K0)), op=OP.subtract)

                # ---- U cross tensors (i,j,k) = (ij, k) ----
                def Bi(t):
                    return bcv(t.rearrange("p (i k) -> p i k", k=K0).unsqueeze(2),
                               (P, K0, K0, K0))

                def Bj(t):
                    return bcv(t.rearrange("p (j k) -> p j k", k=K0).unsqueeze(1),
                               (P, K0, K0, K0))

                U1 = vpool.tile([P, 512], F32, tag="U1")
                U2 = vpool.tile([P, 512], F32, tag="U2")
                U3 = vpool.tile([P, 512], F32, tag="U3")
                uA = vpool.tile([P, 512], F32, tag="uA")
                U1v = U1.rearrange("p (i j k) -> p i j k", j=K0, k=K0)
                U2v = U2.rearrange("p (i j k) -> p i j k", j=K0, k=K0)
                U3v = U3.rearrange("p (i j k) -> p i j k", j=K0, k=K0)
                uAv = uA.rearrange("p (i j k) -> p i j k", j=K0, k=K0)
                VE.tensor_tensor(out=U1v, in0=Bi(by), in1=Bj(bs), op=OP.mult)
                GP.tensor_tensor(out=uAv, in0=Bi(bs), in1=Bj(by), op=OP.mult)
                VE.tensor_tensor(out=U1, in0=U1, in1=uA, op=OP.subtract)
                GP.tensor_tensor(out=U2v, in0=Bi(bx), in1=Bj(bs), op=OP.mult)
                VE.tensor_tensor(out=uAv, in0=Bi(bs), in1=Bj(bx), op=OP.mult)
                GP.tensor_tensor(out=U2, in0=U2, in1=uA, op=OP.subtract)
                VE.tensor_tensor(out=U3v, in0=Bi(bx), in1=Bj(by), op=OP.mult)
                GP.tensor_tensor(out=uAv, in0=Bi(by), in1=Bj(bx), op=OP.mult)
                VE.tensor_tensor(out=U3, in0=U3, in1=uA, op=OP.subtract)

                # ---- dets E(ij, c, k) = bx*U1 - by*U2 + bs*U3; keep iff
                #      max_k E <= 0 (== baseline det' >= 0 with v0 = -b) ----
                def Uv(t):
                    return bcv(t.rearrange("p (q k) -> p q k", k=K0).unsqueeze(2),
                               (P, K2, K0, K0))

                def Bc(t):
                    return bcv(t.rearrange("p (c k) -> p c k", k=K0).unsqueeze(1),
                               (P, K2, K0, K0))

                e1 = dpool.tile([P, DET], F32, tag="e1")
                e2 = dpool.tile([P, DET], F32, tag="e2")
                e1v = e1.rearrange("p (q c k) -> p q c k", c=K0, k=K0)
                e2v = e2.rearrange("p (q c k) -> p q c k", c=K0, k=K0)
                VE.tensor_tensor(out=e1v, in0=Uv(U1), in1=Bc(bx), op=OP.mult)
                GP.tensor_tensor(out=e2v, in0=Uv(U2), in1=Bc(by), op=OP.mult)
                VE.tensor_tensor(out=e1, in0=e1, in1=e2, op=OP.subtract)
                GP.tensor_tensor(out=e2v, in0=Uv(U3), in1=Bc(bs), op=OP.mult)
                VE.tensor_tensor(out=e1, in0=e1, in1=e2, op=OP.add)
                maxE = vpool.tile([P, 512], F32, tag="maxE")
                maxEv = maxE.rearrange("p (q c) -> p q c", c=K0)
                VE.tensor_reduce(out=maxEv, in_=e1v, axis=AX.X, op=OP.max)
                # okb = (maxE <= 0) * neq(i,j,c) * 2^c ; okall = sum_c
                okb = vpool.tile([P, 512], F32, tag="okb")
                mskE = vpool.tile([P, 512], F32, tag="mskE")
                VE.tensor_scalar(out=mskE, in0=maxE, scalar1=0.0, scalar2=None,
                                 op0=OP.is_le)
                GP.tensor_tensor(out=okb, in0=mskE, in1=NEQP, op=OP.mult)
                okall = vpool.tile([P, K2], F32, tag="okall")
                VE.tensor_reduce(out=okall,
                                 in_=okb.rearrange("p (q c) -> p q c", c=K0),
                                 axis=AX.X, op=OP.add)

                # ---- per-r: distances / closest ----
                px_rk = bcv(px.unsqueeze(1), (P, RA, K0))
                py_rk = bcv(py.unsqueeze(1), (P, RA, K0))
                tx_rk = bcv(TX.unsqueeze(2), (P, RA, K0))
                ty_rk = bcv(TY.unsqueeze(2), (P, RA, K0))
                tdx = rkpool.tile([P, RK], F32, tag="tdx")
                tdy = rkpool.tile([P, RK], F32, tag="tdy")
                d2 = rkpool.tile([P, RK], F32, tag="d2")
                tdxv = tdx.rearrange("p (r k) -> p r k", k=K0)
                tdyv = tdy.rearrange("p (r k) -> p r k", k=K0)
                d2v = d2.rearrange("p (r k) -> p r k", k=K0)
                VE.tensor_tensor(out=tdxv, in0=px_rk, in1=tx_rk, op=OP.subtract)
                GP.tensor_tensor(out=tdyv, in0=py_rk, in1=ty_rk, op=OP.subtract)
                SC.activation(out=tdx, in_=tdx, func=AF.Square)
                SC.activation(out=tdy, in_=tdy, func=AF.Square)
                VE.tensor_tensor(out=d2, in0=tdx, in1=tdy, op=OP.add)
                dmin = opool.tile([P, RA], F32, tag="dmin")
                VE.tensor_reduce(out=dmin, in_=d2v, axis=AX.X, op=OP.min)
                dmin_rk = bcv(dmin.unsqueeze(2), (P, RA, K0))
                m0 = rkpool.tile([P, RK], F32, tag="m0")
                m0v = m0.rearrange("p (r k) -> p r k", k=K0)
                VE.tensor_tensor(out=m0v, in0=d2v, in1=dmin_rk, op=OP.is_equal)

                # gather streams: cx, cy, c_f (iota8)
                STRM = vpool.tile([P, 24], F32, tag="STRM")
                SC.copy(out=STRM[:, 0:8], in_=px)
                SC.copy(out=STRM[:, 8:16], in_=py)
                SC.copy(out=STRM[:, 16:24], in_=IOTA8)
                gm = rkpool.tile([P, RK * 3], F32, tag="gm")
                gmv = gm.rearrange("p (r g k) -> p r g k", g=3, k=K0)
                GP.tensor_tensor(
                    out=gmv, in0=bcv(m0v.unsqueeze(2), (P, RA, 3, K0)),
                    in1=bcv(STRM.rearrange("p (g k) -> p g k", k=K0).unsqueeze(1),
                            (P, RA, 3, K0)), op=OP.mult)
                g4 = opool.tile([P, RA * 3], F32, tag="g4")
                g4v = g4.rearrange("p (r g) -> p r g", g=3)
                VE.tensor_reduce(out=g4v, in_=gmv, axis=AX.X, op=OP.add)
                cx = g4v[:, :, 0:1].squeeze(2)
                cy = g4v[:, :, 1:2].squeeze(2)
                c_f = g4v[:, :, 2:3].squeeze(2)

                # second closest (for fallback index)
                d2b = rkpool.tile([P, RK], F32, tag="d2b")
                VE.scalar_tensor_tensor(out=d2b, in0=m0, scalar=BIG, in1=d2,
                                        op0=OP.mult, op1=OP.add)
                dmin2 = opool.tile([P, RA], F32, tag="dmin2")
                d2bv = d2b.rearrange("p (r k) -> p r k", k=K0)
                VE.tensor_reduce(out=dmin2, in_=d2bv, axis=AX.X, op=OP.min)
                eq2 = rkpool.tile([P, RK], F32, tag="eq2")
                eq2v = eq2.rearrange("p (r k) -> p r k", k=K0)
                VE.tensor_tensor(out=eq2v, in0=d2bv,
                                 in1=bcv(dmin2.unsqueeze(2), (P, RA, K0)),
                                 op=OP.is_equal)
                GP.tensor_tensor(out=eq2v, in0=eq2v,
                                 in1=bcv(IOTA8.unsqueeze(1), (P, RA, K0)),
                                 op=OP.mult)
                o1_f = opool.tile([P, RA], F32, tag="o1_f")
                VE.tensor_reduce(out=o1_f, in_=eq2v, axis=AX.X, op=OP.add)

                # ---- v0, d00, d02 ----
                v0x = rkpool.tile([P, RK], F32, tag="v0x")
                v0y = rkpool.tile([P, RK], F32, tag="v0y")
                v0xv = v0x.rearrange("p (r k) -> p r k", k=K0)
                v0yv = v0y.rearrange("p (r k) -> p r k", k=K0)
                VE.tensor_tensor(out=v0xv, in0=px_rk,
                                 in1=bcv(cx.unsqueeze(2), (P, RA, K0)),
                                 op=OP.subtract)
                GP.tensor_tensor(out=v0yv, in0=py_rk,
                                 in1=bcv(cy.unsqueeze(2), (P, RA, K0)),
                                 op=OP.subtract)
                q1 = rkpool.tile([P, RK], F32, tag="q1")
                q2 = rkpool.tile([P, RK], F32, tag="q2")
                d00 = rkpool.tile([P, RK], F32, tag="d00")
                SC.activation(out=q1, in_=v0x, func=AF.Square)
                SC.activation(out=q2, in_=v0y, func=AF.Square)
                VE.tensor_tensor(out=d00, in0=q1, in1=q2, op=OP.add)

                v2x = opool.tile([P, RA], F32, tag="v2x")
                v2y = opool.tile([P, RA], F32, tag="v2y")
                VE.tensor_tensor(out=v2x, in0=TX, in1=cx, op=OP.subtract)
                VE.tensor_tensor(out=v2y, in0=TY, in1=cy, op=OP.subtract)
                d02 = rkpool.tile([P, RK], F32, tag="d02")
                d02v = d02.rearrange("p (r k) -> p r k", k=K0)
                VE.tensor_tensor(out=q1.rearrange("p (r k) -> p r k", k=K0),
                                 in0=v0xv, in1=bcv(v2x.unsqueeze(2), (P, RA, K0)),
                                 op=OP.mult)
                GP.tensor_tensor(out=q2.rearrange("p (r k) -> p r k", k=K0),
                                 in0=v0yv, in1=bcv(v2y.unsqueeze(2), (P, RA, K0)),
                                 op=OP.mult)
                VE.tensor_tensor(out=d02, in0=q1, in1=q2, op=OP.add)

                # ---- (r, i, j) chain ----
                def XI(t2):
                    return bcv(t2.rearrange("p (r k) -> p r k", k=K0).unsqueeze(3),
                               (P, RA, K0, K0))

                def XJ(t2):
                    return bcv(t2.rearrange("p (r k) -> p r k", k=K0).unsqueeze(2),
                               (P, RA, K0, K0))

                def T4(t):
                    return t.rearrange("p (r i j) -> p r i j", i=K0, j=K0)

                def T4T(t):
                    return t.rearrange("p (r j i) -> p r i j", j=K0, i=K0)

                s0 = ipool.tile([P, PP], F32, tag="s0")
                s1 = ipool.tile([P, PP], F32, tag="s1")
                s2 = ipool.tile([P, PP], F32, tag="s2")
                s3 = ipool.tile([P, PP], F32, tag="s3")
                s4 = ipool.tile([P, PP], F32, tag="s4")
                s5 = ipool.tile([P, PP], F32, tag="s5")
                s6 = ipool.tile([P, PP], F32, tag="s6")
                w2 = ipool.tile([P, PP], F32, tag="w2")
                si = ipool.tile([P, PP], I16, tag="si")

                GP.tensor_tensor(out=T4(s0), in0=XI(v0x), in1=XJ(v0x), op=OP.mult)
                GP.tensor_tensor(out=T4(s1), in0=XI(v0y), in1=XJ(v0y), op=OP.mult)
                VE.tensor_tensor(out=s2, in0=s0, in1=s1, op=OP.add)  # dot01
                GP.tensor_tensor(out=T4(s0), in0=XI(d00), in1=XJ(d00), op=OP.mult)
                SC.activation(out=s1, in_=s2, func=AF.Square)        # dot01^2
                VE.tensor_tensor(out=s3, in0=s0, in1=s1, op=OP.subtract)  # denom
                VE.reciprocal_approx_accurate(out=s4, in_=s3, scratch=s5)
                VE.tensor_scalar(out=s5, in0=s4, scalar1=BIG, scalar2=-BIG,
                                 op0=OP.min, op1=OP.max)             # inv clamped
                VE.tensor_tensor(out=T4(s0), in0=XJ(d00), in1=XI(d02), op=OP.mult)
                GP.tensor_tensor(out=T4(s1), in0=T4(s2), in1=XJ(d02), op=OP.mult)
                VE.tensor_tensor(out=s3, in0=s0, in1=s1, op=OP.subtract)  # num
                GP.tensor_tensor(out=w2, in0=s3, in1=s5, op=OP.mult)      # w2
                # w1 = transpose(w2); w0 = 1 - w2 - w1
                GP.tensor_tensor(out=T4(s1), in0=T4(w2), in1=T4T(w2), op=OP.add)
                SC.activation(out=s2, in_=s1, func=AF.Copy, bias=1.0, scale=-1.0)
                # min/max(w2, w2T) via shared relu: d=w2T-w2, r=relu(d)
                #   min = w2T - r ; max = w2 + r ; then combine with w0 on VE
                GP.tensor_tensor(out=T4(s1), in0=T4T(w2), in1=T4(w2), op=OP.subtract)
                SC.activation(out=s1, in_=s1, func=AF.Relu)
                GP.tensor_tensor(out=T4(s3), in0=T4T(w2), in1=T4(s1), op=OP.subtract)
                GP.tensor_tensor(out=T4(s6), in0=T4(w2), in1=T4(s1), op=OP.add)
                VE.tensor_tensor(out=s1, in0=s6, in1=s2, op=OP.max)
                VE.tensor_tensor(out=s6, in0=s3, in1=s2, op=OP.min)  # wmin
                s3, s1 = s1, s3                                      # wmax in s3

                # ---- okbit extract: (okall_int >> c) & 1 -> float ----
                # all casts are integral-valued (no trunc-vs-round ambiguity)
                oki = vpool.tile([P, K2], I16, tag="oki")
                VE.tensor_copy(out=oki, in_=okall)
                ci = opool.tile([P, RA], I16, tag="ci")
                VE.tensor_copy(out=ci, in_=c_f)
                VE.tensor_tensor(out=si.rearrange("p (r q) -> p r q", q=K2),
                                 in0=bcv(oki.unsqueeze(1), (P, RA, K2)),
                                 in1=bcv(ci.unsqueeze(2), (P, RA, K2)),
                                 op=OP.logical_shift_right)
                VE.tensor_scalar(out=si, in0=si, scalar1=1, scalar2=None,
                                 op0=OP.bitwise_and)
                SC.copy(out=s1, in_=si)                               # bitf
                # valid = (wmin > 0) * bitf ; score = max(wmax, (1-valid)*BIG)
                VE.scalar_tensor_tensor(out=s2, in0=s6, scalar=0.0, in1=s1,
                                        op0=OP.is_gt, op1=OP.mult)
                SC.activation(out=s1, in_=s2, func=AF.Copy, bias=BIG, scale=-BIG)
                VE.tensor_tensor(out=s2, in0=s3, in1=s1, op=OP.max)   # score
                smin = opool.tile([P, RA], F32, tag="smin")
                VE.tensor_reduce(out=smin,
                                 in_=s2.rearrange("p (r q) -> p r q", q=K2),
                                 axis=AX.X, op=OP.min)
                VE.tensor_tensor(out=s1.rearrange("p (r q) -> p r q", q=K2),
                                 in0=s2.rearrange("p (r q) -> p r q", q=K2),
                                 in1=bcv(smin.unsqueeze(2), (P, RA, K2)),
                                 op=OP.is_equal)                      # oh0
                VE.scalar_tensor_tensor(out=s3.rearrange("p (r q) -> p r q", q=K2),
                                        in0=s1.rearrange("p (r q) -> p r q", q=K2),
                                        scalar=-SB,
                                        in1=bcv(IOTA64B.unsqueeze(1), (P, RA, K2)),
                                        op0=OP.mult, op1=OP.add)      # pidt
                pidx = opool.tile([P, RA], F32, tag="pidx")
                VE.tensor_reduce(out=pidx,
                                 in_=s3.rearrange("p (r q) -> p r q", q=K2),
                                 axis=AX.X, op=OP.min)
                # gathers: w2sel = sum oh0*w2 ; w1sel = sum oh0*w2^T
                G = ipool.tile([P, 2 * PP], F32, tag="G")
                VE.tensor_tensor(out=G[:, 0:PP], in0=s1, in1=w2, op=OP.mult)
                GP.tensor_tensor(out=T4(G[:, PP:2 * PP]),
                                 in0=T4(s1), in1=T4T(w2), op=OP.mult)
                gsel = opool.tile([P, 2 * RA], F32, tag="gsel")
                VE.tensor_reduce(out=gsel.rearrange("p (g r) -> p g r", g=2),
                                 in_=G.rearrange("p (g r q) -> p g r q",
                                                 g=2, q=K2),
                                 axis=AX.X, op=OP.add)
                w2sel = gsel[:, 0:RA]
                w1sel = gsel[:, RA:2 * RA]

                # ---- tail: fallback, index decode, outputs ----
                fb = opool.tile([P, RA], F32, tag="fb")
                nfb = opool.tile([P, RA], F32, tag="nfb")
                VE.tensor_scalar(out=fb, in0=smin, scalar1=1.0e38, scalar2=None,
                                 op0=OP.is_ge)
                VE.tensor_scalar(out=nfb, in0=fb, scalar1=-1.0, scalar2=1.0,
                                 op0=OP.mult, op1=OP.add)
                w2o = opool.tile([P, RA], F32, tag="w2o")
                w1o = opool.tile([P, RA], F32, tag="w1o")
                w0o = opool.tile([P, RA], F32, tag="w0o")
                VE.tensor_tensor(out=w2o, in0=w2sel, in1=nfb, op=OP.mult)
                VE.tensor_tensor(out=w1o, in0=w1sel, in1=nfb, op=OP.mult)
                VE.tensor_tensor(out=w0o, in0=w2o, in1=w1o, op=OP.add)
                VE.tensor_scalar(out=w0o, in0=w0o, scalar1=-1.0, scalar2=1.0,
                                 op0=OP.mult, op1=OP.add)
                VE.tensor_tensor(out=w0o, in0=w0o, in1=nfb, op=OP.mult)

                pidxi = opool.tile([P, RA], I32, tag="pidxi")
                i_i = opool.tile([P, RA], I32, tag="i_i")
                j_i = opool.tile([P, RA], I32, tag="j_i")
                i_f = opool.tile([P, RA], F32, tag="i_f")
                j_f = opool.tile([P, RA], F32, tag="j_f")
                VE.tensor_copy(out=pidxi, in_=pidx)
                VE.tensor_scalar(out=i_i, in0=pidxi, scalar1=3, scalar2=None,
                                 op0=OP.arith_shift_right)
                VE.tensor_scalar(out=j_i, in0=pidxi, scalar1=7, scalar2=None,
                                 op0=OP.bitwise_and)
                VE.tensor_copy(out=i_f, in_=i_i)
                VE.tensor_copy(out=j_f, in_=j_i)
                VE.copy_predicated(out=i_f, mask=fb.bitcast(I32), data=o1_f)
                VE.copy_predicated(out=j_f, mask=fb.bitcast(I32), data=o1_f)

                wout = opool.tile([P, RA * 3], F32, tag="wout")
                iout = opool.tile([P, RA * 3], F32, tag="iout")
                woutv = wout.rearrange("p (r c) -> p r c", c=3)
                ioutv = iout.rearrange("p (r c) -> p r c", c=3)
                SC.copy(out=woutv[:, :, 0], in_=w0o)
                SC.copy(out=woutv[:, :, 1], in_=w2o)
                SC.copy(out=woutv[:, :, 2], in_=w1o)
                SC.copy(out=ioutv[:, :, 0], in_=c_f)
                SC.copy(out=ioutv[:, :, 1], in_=i_f)
                SC.copy(out=ioutv[:, :, 2], in_=j_f)
                nc.sync.dma_start(outw_d[v0_:v1_, :, :], woutv)
                nc.sync.dma_start(outi_d[v0_:v1_, :, :], ioutv)

    nc.compile()
    return nc


def make_consts():
    i = np.arange(K0)
    ii = (np.arange(K2) // K0)[:, None]
    jj = (np.arange(K2) % K0)[:, None]
    cc = i[None, :]
    neq = (ii != jj) & (ii != cc) & (jj != cc)          # (64, 8)
    neqp = (neq * (2.0 ** cc)).astype(np.float32).reshape(1, 512)
    iota8 = i.astype(np.float32).reshape(1, K0)
    p2neg = (2.0 ** (-i)).astype(np.float32).reshape(1, K0)
    iota64b = (np.arange(K2) + SB).astype(np.float32).reshape(1, K2)
    return {"neqp": neqp, "iota8": iota8, "p2neg": p2neg, "iota64b": iota64b}


def make_in_maps(template, projections):
    template = np.ascontiguousarray(np.asarray(template, np.float32))
    projections = np.ascontiguousarray(np.asarray(projections, np.float32))
    consts = make_consts()
    tmplT = np.stack([template[..., 0].reshape(-1), template[..., 1].reshape(-1)])
    px_all = np.ascontiguousarray(projections[..., 0])
    py_all = np.ascontiguousarray(projections[..., 1])
    in_maps = []
    for c in range(N_CORES):
        pxc = px_all[c * VS:(c + 1) * VS]
        pyc = py_all[c * VS:(c + 1) * VS]
        pad = VSP - VS
        pxc = np.concatenate([pxc, np.broadcast_to(pxc[:1], (pad, K0))], 0)
        pyc = np.concatenate([pyc, np.broadcast_to(pyc[:1], (pad, K0))], 0)
        m = {"px": np.ascontiguousarray(pxc), "py": np.ascontiguousarray(pyc),
             "tmpl": tmplT}
        m.update(consts)
        in_maps.append(m)
    return in_maps


_NC_CACHE = {}


def kernel(template, projections, _want_time=False):
    from concourse.bass_utils import run_bass_kernel_spmd
    if "nc" not in _NC_CACHE:
        _NC_CACHE["nc"] = build_nc()
    nc = _NC_CACHE["nc"]
    in_maps = make_in_maps(template, projections)
    res = run_bass_kernel_spmd(nc, in_maps, core_ids=list(range(N_CORES)))
    ws, idxs = [], []
    for c in range(N_CORES):
        out = res.results[c]
        ws.append(out["outw"][:VS].reshape(VS, R, A, 3))
        idxs.append(out["outi"][:VS].reshape(VS, R, A, 3))
    w = np.concatenate(ws, 0).astype(np.float32)
    idx = np.rint(np.concatenate(idxs, 0)).astype(np.int32)
    if _want_time:
        return (w, idx), res
    return w, idx
